# revision 24
# baseline (speedup 1.0000x reference)
"""GAT (3-layer, PyG GATConv-style) Trainium2 Bass kernel, 8-core SPMD.

Instruction-count-optimized rewrite. The axon execution path costs ~constant
time per instruction, so the kernel is organized to touch as many elements as
possible per instruction:

  - Nodes are permuted by in-degree and dealt round-robin to the 8 cores, so
    each 128-dst "block" has near-uniform in-degree. Incoming edges of block b
    are packed into a dense [128 dst-slot, KB_b] grid (k-major), padded with a
    dedicated -inf row so padded slots contribute exp(-large)=0.
  - Per block, ONE dma_gather chunk pulls up to 1024 src rows (features +
    fused attention scores) into [128, kn, RW]; attention + softmax numer/
    denom are computed with ~6 wide vector ops per block using broadcast
    access patterns and free-dim tensor_reduce (no per-tile matmuls).
  - Segment softmax normalization, head-mean, bias, relu are batched across
    all 20 blocks in ~7 instructions; the next-layer transposed input is
    produced by a single DMA-transpose (XBAR).
  - Layer tables are AllGathered once per layer in a single collective.

Node bookkeeping: within a core, node t = b*128 + s (block b, slot s) is
column t of xT/x2T and row t of the local hl table; its global hf row is
core*ROWPAD + t. The XBAR DMA-transpose is a blockwise 128-column panel
transpose: out[p, j, s] = in[s, j*128 + p] for a 3D [128, B, 128] out AP.
"""

import numpy as np
import ml_dtypes

BF16 = ml_dtypes.bfloat16
NCORES = 8
PADR = 16          # -inf pad rows appended to each core's hl table
GC = 8             # k-slices per gather (8*128 = 1024 idxs)
SCW = 16           # score column block width (H src + H dst + zeros)
PAIR_CAP = 72      # max 128-slices in a paired gather tile (SBUF budget)


# ----------------------------------------------------------------------------
# Host-side preprocessing
# ----------------------------------------------------------------------------

def _wrap16(idx_flat):
    """dma_gather index layout: [128, n/16] int16, idx i at [i%16, i//16],
    replicated across the 8 groups of 16 partitions."""
    n = idx_flat.shape[0]
    assert n % 16 == 0
    w = idx_flat.reshape(n // 16, 16).T.astype(np.int16)  # [16, n/16]
    return np.tile(w, (8, 1))  # [128, n/16]


def prep_static(edge_index, N, NPAD):
    """Degree-balanced node permutation + dense per-block gather tables.

    Returns (KB, idx_cores, perm) where perm[c] lists original node ids in
    core-c slot order t=0..NPC-1 (pad slots = -1)."""
    NPC = NPAD // NCORES
    B = NPC // 128
    ROWPAD = NPC + PADR

    E0 = edge_index.shape[1]
    src0 = edge_index[0].astype(np.int64)
    dst0 = edge_index[1].astype(np.int64)
    deg = np.zeros(NPAD, dtype=np.int64)
    np.add.at(deg, dst0, 1)
    deg[:N] += 1  # self-loop
    order = np.argsort(-deg, kind="stable")      # positions -> orig node
    pos = np.empty(NPAD, dtype=np.int64)
    pos[order] = np.arange(NPAD)
    core_of = pos % NCORES
    slot_of = pos // NCORES                      # t within core (block-major)

    # per-block k-capacity: max degree among the block's 1024 sorted positions
    KB = [int(max(1, deg[order[b * 128 * NCORES]])) for b in range(B)]
    # group adjacent blocks into pairs where the paired gather tile stays small
    groups = []   # (b0, gsz, KBG)
    b = 0
    while b < B:
        if b + 1 < B and 2 * max(KB[b], KB[b + 1]) <= PAIR_CAP:
            groups.append((b, 2, max(KB[b], KB[b + 1])))
            b += 2
        else:
            groups.append((b, 1, KB[b]))
            b += 1
    # per-block placement within its group's k-major interleaved table
    goffb = np.zeros(B, dtype=np.int64)
    gszb = np.zeros(B, dtype=np.int64)
    jb = np.zeros(B, dtype=np.int64)
    goff = 0
    for (b0, gsz, KBG) in groups:
        for j in range(gsz):
            goffb[b0 + j] = goff
            gszb[b0 + j] = gsz
            jb[b0 + j] = j
        goff += KBG * gsz * 128
    SUMSL = goff // 128   # total 128-slices

    # row index within core table = slot t (block-major: t = b*128 + s)
    row_of = core_of * ROWPAD + slot_of          # global hf row per node
    PADROW = NPC                                  # core 0's first -inf row

    # edges incl self-loops, grouped per destination
    src = np.concatenate([src0, np.arange(N, dtype=np.int64)])
    dst = np.concatenate([dst0, np.arange(N, dtype=np.int64)])
    key = core_of[dst] * NPC + slot_of[dst]
    eorder = np.argsort(key, kind="stable")
    key_s = key[eorder]
    srcrow_s = row_of[src[eorder]]
    cnt = np.bincount(key_s, minlength=NCORES * NPC)
    run_start = np.zeros(NCORES * NPC, dtype=np.int64)
    run_start[1:] = np.cumsum(cnt)[:-1]
    k_e = np.arange(len(key_s)) - run_start[key_s]

    c_e = key_s // NPC
    t_e = key_s % NPC
    b_e = t_e // 128
    s_e = t_e % 128
    assert (k_e < np.array(KB)[b_e]).all()

    idx_cores = []
    for c in range(NCORES):
        tab = np.full(goff, PADROW, dtype=np.int64)
        m = c_e == c
        pos = (goffb[b_e[m]] + (k_e[m] * gszb[b_e[m]] + jb[b_e[m]]) * 128
               + s_e[m])
        tab[pos] = srcrow_s[m]
        idx_cores.append(_wrap16(tab))
    perm = [order[np.arange(NPC) * NCORES + c] for c in range(NCORES)]
    perm = [np.where(p < N, p, -1) for p in perm]
    return tuple(groups), SUMSL, idx_cores, perm


def _row_width(H, C):
    """table row width: H*C features + SCW score block, padded to 128 elems."""
    used = H * C + SCW
    return ((used + 127) // 128) * 128


def prep_values(x, Ws, a_srcs, a_dsts, perm_c, B):
    """Per-core xT (permuted to r=s*B+b column order) + fused W tables."""
    N, F = x.shape
    NPC = perm_c[0].shape[0]
    xTs = []
    for p in perm_c:
        xp = np.zeros((NPC, F), dtype=np.float32)
        ok = p >= 0
        xp[ok] = x[p[ok]]
        xTs.append(np.ascontiguousarray(xp.T).astype(BF16))  # [F, NPC]

    W_augs = []
    for W, a_s, a_d in zip(Ws, a_srcs, a_dsts):
        H, Fin, C = W.shape
        RW = _row_width(H, C)
        Wf = np.transpose(W, (1, 0, 2)).reshape(Fin, H * C)
        wsrc = np.einsum("hfc,hc->fh", W, a_s)
        wdst = np.einsum("hfc,hc->fh", W, a_d)
        Wa = np.zeros((Fin, RW), dtype=np.float32)
        Wa[:, : H * C] = Wf
        Wa[:, H * C : H * C + H] = wsrc
        Wa[:, H * C + H : H * C + 2 * H] = wdst
        W_augs.append(Wa.astype(BF16))
    return xTs, W_augs


# ----------------------------------------------------------------------------
# Device program
# ----------------------------------------------------------------------------

def build_nc(cfg, repeat=1):
    import concourse.bacc as bacc
    import concourse.mybir as mybir
    import concourse.tile as tile
    from contextlib import ExitStack

    f32 = mybir.dt.float32
    bf16 = mybir.dt.bfloat16
    i16 = mybir.dt.int16
    ALU = mybir.AluOpType
    ACT = mybir.ActivationFunctionType

    N = cfg["N"]
    NPAD = cfg["NPAD"]
    F_IN = cfg["F_IN"]
    C = cfg["C"]
    GROUPS = cfg["GROUPS"]
    SUMSL = cfg["SUMSL"]
    HS = cfg["HS"]
    NPC = NPAD // NCORES
    B = NPC // 128
    ROWPAD = NPC + PADR
    NL = len(HS)
    RWs = [_row_width(H, C) for H in HS]
    FINs = [F_IN] + [C] * (NL - 1)
    goff = [0]
    for (_, gsz, KBG) in GROUPS:
        goff.append(goff[-1] + KBG * gsz)

    nc = bacc.Bacc("TRN2", target_bir_lowering=False, debug=False,
                   num_devices=NCORES)

    # ---- I/O ----
    xT_d = nc.dram_tensor("xT", [F_IN, NPC], bf16, kind="ExternalInput")
    idx_d = nc.dram_tensor("idx", [128, SUMSL * 8], i16, kind="ExternalInput")
    W_d = [nc.dram_tensor(f"w{i+1}", [FINs[i], RWs[i]], bf16,
                          kind="ExternalInput") for i in range(NL)]
    bb_d = [nc.dram_tensor(f"bb{i+1}", [128, C], f32, kind="ExternalInput")
            for i in range(NL - 1)]
    b3r_d = nc.dram_tensor("b3r", [1, C], f32, kind="ExternalInput")
    out_d = nc.dram_tensor("out", [1, C], f32, kind="ExternalOutput")

    with tile.TileContext(nc, num_cores=NCORES) as tc, ExitStack() as ctx:
        dram = ctx.enter_context(tc.tile_pool(name="dram", bufs=1, space="DRAM"))
        cpool = ctx.enter_context(tc.tile_pool(name="consts", bufs=1))
        gpool = ctx.enter_context(tc.tile_pool(name="gath", bufs=1))
        wpool = ctx.enter_context(tc.tile_pool(name="work", bufs=1))
        psum = ctx.enter_context(tc.tile_pool(name="ps", bufs=1, space="PSUM"))

        hl = [dram.tile([ROWPAD, RWs[i]], bf16, tag=f"hl{i}", name=f"hl{i}")
              for i in range(NL)]
        hf = [dram.tile([ROWPAD * NCORES, RWs[i]], bf16, tag=f"hf{i}",
                        name=f"hf{i}") for i in range(NL)]

        # ---- constants into SBUF (outside the timed repeat loop) ----
        xT_sb = cpool.tile([F_IN, NPC], bf16, tag="xT")
        nc.sync.dma_start(xT_sb[:], xT_d[:, :])
        idx_sb = cpool.tile([128, SUMSL * 8], i16, tag="idx")
        nc.sync.dma_start(idx_sb[:], idx_d[:, :])
        W_sb = []
        for i in range(NL):
            w = cpool.tile([FINs[i], RWs[i]], bf16, tag=f"w{i}", name=f"w{i}")
            nc.sync.dma_start(w[:], W_d[i][:, :])
            W_sb.append(w)
        bb_sb = []
        for i in range(NL - 1):
            b = cpool.tile([128, C], f32, tag=f"bb{i}", name=f"bb{i}")
            nc.sync.dma_start(b[:], bb_d[i][:, :])
            bb_sb.append(b)
        b3_sb = cpool.tile([1, C], f32, tag="b3")
        nc.sync.dma_start(b3_sb[:], b3r_d[:, :])
        ones_sb = cpool.tile([128, 1], f32, tag="ones")
        nc.vector.memset(ones_sb[:], 1.0)
        # -inf pad rows (scores -> exp ~ 0; features multiplied by 0)
        ninf = cpool.tile([PADR, max(RWs)], bf16, tag="ninf")
        nc.vector.memset(ninf[:], -30000.0)
        for i in range(NL):
            nc.sync.dma_start(hl[i][NPC:ROWPAD, :], ninf[:, 0:RWs[i]])

        # next-layer transposed features (single buffer shared by both uses)
        x2T = cpool.tile([128, NPC], bf16, tag="x2T")

        # hoisted num_idxs registers for the gathers (one per distinct value)
        kns = sorted({min(GC, gsz * KBG - k0) for (_, gsz, KBG) in GROUPS
                      for k0 in range(0, gsz * KBG, GC)})
        kn_regs = {kn: nc.gpsimd.to_reg(kn * 128) for kn in kns}

        for _rep in range(repeat):
         for L in range(NL):
            H = HS[L]
            RW = RWs[L]
            FIN = FINs[L]
            HC = H * C
            HCF = min(HC, 512)   # feature matmul width (bank-limited)

            # ---- phase A: h_aug rows for own nodes ----
            lhs = xT_sb if L == 0 else x2T

            # scores (transposed): scb[j, t] for j in [0, SCW)
            scb = wpool.tile([SCW, NPC], bf16, tag="scb")
            nchk = NPC // 512                     # 512-node score chunks
            for g0 in range(0, nchk, 3):
                gn = min(3, nchk - g0)
                psc = psum.tile([SCW, 3, 512], f32, tag="psc")
                for j in range(gn):
                    nc.tensor.matmul(
                        psc[:, j, :], W_sb[L][:, HC:HC + SCW],
                        lhs[:, (g0 + j) * 512:(g0 + j + 1) * 512],
                        start=True, stop=True)
                nc.scalar.copy(scb[:, g0 * 512:(g0 + gn) * 512],
                               psc[:, 0:gn, :])

            # features: 128-node chunks, groups of 4 PSUM banks; assemble
            # rows in a small rotating staging tile and DMA per group
            for g0 in range(0, B, 4):
                gn = min(4, B - g0)
                pf = psum.tile([128, 4, 512], f32, tag="pf")
                for j in range(gn):
                    nc.tensor.matmul(
                        pf[:, j, 0:HCF], lhs[:, (g0 + j) * 128:(g0 + j + 1) * 128],
                        W_sb[L][:, 0:HCF], start=True, stop=True)
                stg = wpool.tile([128, 4, RW], bf16, tag="stage")
                nc.scalar.copy(stg[:, 0:gn, 0:HCF], pf[:, 0:gn, 0:HCF])
                nc.sync.dma_start(stg[:, 0:gn, HC:HC + SCW],
                                  scb[:, g0 * 128:(g0 + gn) * 128],
                                  transpose=True)
                nc.sync.dma_start(
                    hl[L][g0 * 128:(g0 + gn) * 128, :].rearrange(
                        "(ch p) w -> p ch w", p=128),
                    stg[:, 0:gn, :])

            # ---- phase B: allgather ----
            nc.gpsimd.collective_compute(
                "AllGather", mybir.AluOpType.bypass,
                replica_groups=[list(range(NCORES))],
                ins=[hl[L][:, :].opt()],
                outs=[hf[L][:, :].opt()],
            )

            # s_dst for own nodes: block b slot s -> row s*B + b
            sdst = wpool.tile([128, B, H], bf16, tag="sdst")
            nc.sync.dma_start(
                sdst[:],
                hl[L][0:NPC, HC + H:HC + 2 * H].rearrange(
                    "(b s) h -> s b h", s=128))

            # accumulators over all blocks
            numer = wpool.tile([128, B, HC], f32, tag="numer")
            denom = wpool.tile([128, B, H], f32, tag="denom")

            # ---- phase C: per dst block group (1 or 2 blocks interleaved) ----
            for gi, (b0, gsz, KBG) in enumerate(GROUPS):
                TS = gsz * KBG                    # total 128-slices
                g1 = gpool.tile([128, TS, RW], bf16, tag="g1")
                for k0 in range(0, TS, GC):
                    kn = min(GC, TS - k0)
                    ic = slice((goff[gi] + k0) * 8, (goff[gi] + k0 + kn) * 8)
                    nc.gpsimd.dma_gather(g1[:, k0:k0 + kn, :], hf[L][:, :],
                                         idx_sb[:, ic], kn * 128, kn_regs[kn],
                                         RW, elem_step=RW)
                gsc = g1[:, :, HC:HC + H].rearrange("q (k j) h -> q k j h",
                                                    j=gsz)
                sc = wpool.tile([128, KBG, gsz, H], f32, tag="sc")
                nc.vector.tensor_tensor(
                    sc[:], gsc,
                    sdst[:, b0:b0 + gsz, :].unsqueeze(1).broadcast_to(
                        [128, KBG, gsz, H]), ALU.add)
                nc.vector.scalar_tensor_tensor(sc[:], sc[:], 0.2, sc[:],
                                               ALU.mult, ALU.max)
                p = wpool.tile([128, KBG, gsz, H], f32, tag="p")
                nc.scalar.activation(p[:], sc[:], ACT.Exp)
                # msg = h_src * p, in place over gathered features
                for j in range(gsz):
                    gfeat = g1[:].rearrange("q (k j) w -> q k j w", j=gsz)[
                        :, :, j, 0:HC].rearrange("q k (h c) -> q k h c", h=H)
                    nc.vector.tensor_tensor(
                        gfeat, gfeat,
                        p[:, :, j, :].unsqueeze(3).broadcast_to(
                            [128, KBG, H, C]), ALU.mult)
                nc.vector.tensor_reduce(
                    numer[:, b0:b0 + gsz, :],
                    g1[:].rearrange("q (k j) w -> q j w k", j=gsz)[
                        :, :, 0:HC, :],
                    mybir.AxisListType.X, ALU.add)
                nc.vector.tensor_reduce(
                    denom[:, b0:b0 + gsz, :],
                    p[:].rearrange("q k j h -> q j h k"),
                    mybir.AxisListType.X, ALU.add)

            # ---- finalize (batched across blocks) ----
            nc.vector.tensor_scalar(denom[:], denom[:], 1e-16 * H, None,
                                    op0=ALU.add)
            rc = wpool.tile([128, B, H], f32, tag="rc")
            nc.vector.reciprocal(rc[:], denom[:])
            nview = numer[:].rearrange("q b (h c) -> q b h c", h=H)
            nc.vector.tensor_tensor(
                nview, nview,
                rc[:].unsqueeze(3).broadcast_to([128, B, H, C]), ALU.mult)
            if L < NL - 1:
                hm = wpool.tile([128, B, C], f32, tag="hm")
                nc.vector.tensor_reduce(
                    hm[:], nview.transpose([0, 1, 3, 2]),
                    mybir.AxisListType.X, ALU.add)
                m2 = wpool.tile([128, B, C], bf16, tag="m2")
                nc.vector.scalar_tensor_tensor(
                    m2[:], hm[:], 1.0 / H,
                    bb_sb[L][:].unsqueeze(1).broadcast_to([128, B, C]),
                    ALU.mult, ALU.add)
                nc.vector.tensor_scalar(m2[:], m2[:], 0.0, None, op0=ALU.max)
                nc.sync.dma_start(
                    x2T[:].rearrange("q (b s) -> q b s", b=B),
                    m2[:], transpose=True)
            else:
                nsum = wpool.tile([128, C], f32, tag="nsum")
                nc.vector.tensor_reduce(
                    nsum[:], numer[:].transpose([0, 2, 1]),
                    mybir.AxisListType.X, ALU.add)
                pfin = psum.tile([1, C], f32, tag="pfin")
                nc.tensor.matmul(pfin[:], ones_sb[:], nsum[:],
                                 start=True, stop=True)
                fs = wpool.tile([1, C], f32, tag="fs")
                nc.vector.scalar_tensor_tensor(fs[:], pfin[:], 1.0 / N,
                                               b3_sb[:], ALU.mult, ALU.add)
                nc.sync.dma_start(out_d[:, :], fs[:])

    nc.compile()
    return nc


# ----------------------------------------------------------------------------
# Entry points
# ----------------------------------------------------------------------------

def make_cfg_and_maps(inputs):
    x = np.asarray(inputs["x"])
    edge_index = np.asarray(inputs["edge_index"])
    N, F_IN = x.shape
    NPAD = ((N + 1023) // 1024) * 1024
    Ws = [np.asarray(inputs[f"W{i}"]) for i in (1, 2, 3)]
    a_srcs = [np.asarray(inputs[f"as{i}"]) for i in (1, 2, 3)]
    a_dsts = [np.asarray(inputs[f"ad{i}"]) for i in (1, 2, 3)]
    bs = [np.asarray(inputs[f"b{i}"]) for i in (1, 2, 3)]
    HS = tuple(W.shape[0] for W in Ws)
    C = Ws[0].shape[2]
    B = NPAD // NCORES // 128

    groups, SUMSL, idx_cores, perm = prep_static(edge_index, N, NPAD)
    xTs, W_augs = prep_values(x, Ws, a_srcs, a_dsts, perm, B)

    cfg = dict(N=N, NPAD=NPAD, F_IN=F_IN, C=C, GROUPS=groups, SUMSL=SUMSL,
               HS=HS)
    in_maps = []
    for c in range(NCORES):
        m = {
            "xT": xTs[c],
            "idx": idx_cores[c],
            "b3r": (bs[2] * (1.0 / NCORES)).reshape(1, C).astype(np.float32),
        }
        for i in range(3):
            m[f"w{i+1}"] = W_augs[i]
        for i in range(2):
            m[f"bb{i+1}"] = np.broadcast_to(
                bs[i].astype(np.float32), (128, C)).copy()
        in_maps.append(m)
    return cfg, in_maps


_NC_CACHE = {}


def _get_nc(cfg, repeat=1):
    key = (repeat,) + tuple(sorted((k, v if not isinstance(v, tuple) else v)
                                   for k, v in cfg.items()))
    if key not in _NC_CACHE:
        _NC_CACHE[key] = build_nc(cfg, repeat=repeat)
    return _NC_CACHE[key]


def run(inputs, trace=False, repeat=1, **kw):
    from concourse.bass_utils import run_bass_kernel_spmd
    cfg, in_maps = make_cfg_and_maps(inputs)
    nc = _get_nc(cfg, repeat=repeat)
    res = run_bass_kernel_spmd(nc, in_maps, core_ids=list(range(NCORES)),
                               trace=trace, **kw)
    out = np.zeros((1, cfg["C"]), dtype=np.float32)
    for r in res.results:
        out += r["out"]
    return out, res


def kernel(**inputs) -> np.ndarray:
    out, _ = run(inputs)
    return out
